# revision 63
# baseline (speedup 1.0000x reference)
"""Trainium2 Bass kernel for nn_LocalAggBlock (KNN + gather + MLP + maxpool).

Math (exact refactoring of the reference):
  y[n,k] = relu(concat[f_n, f_nb-f_n, p_nb-p_n] @ W + b)
         = relu(a_n + gh[idx[n,k]])
  where a_n  = f_n @ (W1-W2) - p_n @ W3          (per query point)
        gh_m = f_m @ W2 + p_m @ W3 + b            (per reference point)
  out[n] = max_k y[n,k] = relu(a_n + max_k gh[idx[n,k]])   (relu/max commute,
           a_n constant over k)

  KNN ranking uses s'[n,m] = 2 p_n . p_m - ||p_m||^2 (larger = closer; the
  ||p_n||^2 term is constant per row and does not change the ranking).

Sharding: 8 cores = (batch b in 0..1) x (quarter of N).  Each core handles
2048 query points against all 8192 points of its batch.

Host<->device traffic over the axon tunnel is the bottleneck (~80ms RTT,
~95MB/s each way), so payloads are quantized and pipelined:
  - feat uploads as int8 (scale folded into W1/W2 host-side): 1MB up
  - W+b upload on core 0 only, once per call, shared by both batch
    dispatches; an on-device AllGather broadcasts them (other cores ship
    cached zero shards: no wire bytes)
  - output downloads as uint8 with a per-row f32 scale: ~1.06MB down
    (error budget: int8 feat + u8-row output simulates to 9.2e-3 rel err
    vs the 2e-2 gate)
  - one dispatch per batch, each touching all 8 cores but uploading real
    data only to its batch's 4 (cached zero shards elsewhere): batch 0's
    output download overlaps batch 1's input upload on the full-duplex
    tunnel, and host pack/unpack overlaps the wire as well
Full-batch tables are rebuilt on device with NeuronLink AllGathers
(coords, gh table, output).
"""

import numpy as np

import concourse.bacc as bacc
import concourse.bass as bass
import concourse.mybir as mybir
import concourse.tile as tile
from concourse.bass import IndirectOffsetOnAxis
from concourse.masks import make_identity

F32 = mybir.dt.float32
I8 = mybir.dt.int8
U8 = mybir.dt.uint8
U32 = mybir.dt.uint32
AF = mybir.ActivationFunctionType
NEG = -3.0e38

B, N, C = 2, 8192, 64
KNN = 16
NCORES = 8
QPC = B * N // NCORES          # queries per core (2048)
CT_ROWS = 96                   # coords^T (3, 2048) packed as (96, 64)
W_ROWS = 132                   # W (131, 64) + b (1, 64); core 0 only
GROUPS4 = [[0, 1, 2, 3], [4, 5, 6, 7]]   # per-batch replica groups
GROUPS8 = [list(range(NCORES))]


def build_kernel(n_refs=N, n_q=QPC):
    n_chunk = n_refs // 512    # ref chunks per query block
    n_qblk = n_q // 128        # query blocks
    n_grp = NCORES // B        # cores per batch

    nc = bacc.Bacc(None, target_bir_lowering=False)
    fblob = nc.dram_tensor("fblob", [n_q, C], I8, kind="ExternalInput")
    cblob = nc.dram_tensor("cblob", [CT_ROWS, 64], F32, kind="ExternalInput")
    wblob = nc.dram_tensor("wblob", [W_ROWS, 64], F32, kind="ExternalInput")
    outq_d = nc.dram_tensor("outq", [NCORES // B * n_q, C], U8,
                            kind="ExternalOutput")
    outs_d = nc.dram_tensor("outs", [NCORES // B * n_q, 1], F32,
                            kind="ExternalOutput")

    ct_loc = nc.dram_tensor("ct_loc", [CT_ROWS, 64], F32, kind="Internal")
    ct_all = nc.dram_tensor("ct_all", [n_grp * CT_ROWS, 64], F32, kind="Internal")
    w_loc = nc.dram_tensor("w_loc", [W_ROWS, 64], F32, kind="Internal")
    w_all = nc.dram_tensor("w_all", [NCORES * W_ROWS, 64], F32, kind="Internal")
    gh_loc = nc.dram_tensor("gh_loc", [n_q, C], F32, kind="Internal")
    gh_d = nc.dram_tensor("gh", [n_refs, C], F32, kind="Internal")
    outq_loc = nc.dram_tensor("outq_loc", [n_q, C], U8, kind="Internal")
    outs_loc = nc.dram_tensor("outs_loc", [n_q, 1], F32, kind="Internal")
    outq_bounce = nc.dram_tensor("outq_bounce", [n_grp * n_q, C], U8,
                                 kind="Internal")
    outs_bounce = nc.dram_tensor("outs_bounce", [n_grp * n_q, 1], F32,
                                 kind="Internal")

    with tile.TileContext(nc) as tc:
        with tc.tile_pool(name="persist", bufs=1) as pp:
            ident = pp.tile([128, 128], F32)
            make_identity(nc, ident[:])

            # --- coords AllGather (issued first; refsT consumes it later) ---
            nc.sync.dma_start(ct_loc[:], cblob[0:CT_ROWS, :])
            nc.gpsimd.collective_compute(
                "AllGather", mybir.AluOpType.bypass, replica_groups=GROUPS4,
                ins=[ct_loc[:]], outs=[ct_all[:]])
            # W+b broadcast: only core 0 uploads them, once per call (the
            # same wblob array feeds both batch dispatches; other cores ship
            # cached zero shards, no wire bytes); gather over all 8 cores
            # and read core 0's rows.
            nc.sync.dma_start(w_loc[:], wblob[:])
            nc.gpsimd.collective_compute(
                "AllGather", mybir.AluOpType.bypass, replica_groups=GROUPS8,
                ins=[w_loc[:]], outs=[w_all[:]])

            # --- weights (W1/W2 arrive pre-scaled by the feat int8 scale) ---
            wa = pp.tile([C, C], F32)
            wb = pp.tile([C, C], F32)
            wd = pp.tile([C, C], F32)     # W1 - W2
            wc = pp.tile([3, C], F32)
            negwc = pp.tile([3, C], F32)
            bsb = pp.tile([1, C], F32)
            ones1 = pp.tile([1, 128], F32)
            neg3 = pp.tile([3, 1], F32)
            nc.sync.dma_start(wa[:], w_all[0:64, :])
            nc.sync.dma_start(wb[:], w_all[64:128, :])
            nc.sync.dma_start(wc[:], w_all[128:131, :])
            nc.sync.dma_start(bsb[:], w_all[131:132, :])
            nc.vector.tensor_sub(wd[:], wa[:], wb[:])
            nc.vector.tensor_scalar_mul(negwc[:], wc[:], -1.0)
            nc.vector.memset(ones1[:], 1.0)
            nc.vector.memset(neg3[:], -1.0)

            # --- local query coords (transposed) ---
            qTraw = pp.tile([3, n_q], F32)      # raw query coords^T
            qT = pp.tile([4, n_q], F32)         # rows 0-2: 2*p_q^T, row 3: ones
            nc.sync.dma_start(
                qTraw[:], cblob[0:CT_ROWS, :].rearrange("(c r) f -> c (r f)", c=3))
            nc.vector.memset(qT[:], 1.0)  # row 3 stays 1.0
            nc.vector.tensor_scalar_mul(qT[0:3, :], qTraw[:], 2.0)

            # --- full-batch ref coords (transposed) from the AllGather ---
            refsT = pp.tile([4, n_refs], F32)   # rows 0-2: p^T, row 3: -||p||^2
            for g in range(n_grp):
                nc.sync.dma_start(
                    refsT[0:3, g * n_q:(g + 1) * n_q],
                    ct_all[g * CT_ROWS:(g + 1) * CT_ROWS, :].rearrange(
                        "(c r) f -> c (r f)", c=3))

            sq = pp.tile([3, n_refs], F32)
            nc.vector.tensor_mul(sq[:], refsT[0:3, :], refsT[0:3, :])

            a_all = pp.tile([128, n_qblk * C], F32)
            normrow = pp.tile([1, n_refs], F32)

            with tc.tile_pool(name="setup_psum", bufs=2, space="PSUM") as sp, \
                 tc.tile_pool(name="setup_sb", bufs=3) as sb:
                # row 3 of refsT: -(x^2+y^2+z^2) via PE partition-reduce
                for ch in range(n_chunk):
                    psum_n = sp.tile([1, 512], F32, tag="n")
                    nc.tensor.matmul(psum_n[:], neg3[:], sq[:, ch * 512:(ch + 1) * 512],
                                     start=True, stop=True)
                    nc.scalar.activation(normrow[0:1, ch * 512:(ch + 1) * 512],
                                         psum_n[:], AF.Copy)
                # compute engines can't start at partition 3; DMA can
                nc.sync.dma_start(refsT[3:4, :], normrow[:])

                # gh_loc[m] = f_m @ W2 + p_m @ W3 + b  and
                # a[n]      = f_n @ (W1-W2) - p_n @ W3   (same rows; share f^T)
                for rb in range(n_qblk):
                    r0 = rb * 128
                    fblk = sb.tile([128, C], I8, tag="fblk")
                    nc.sync.dma_start(fblk[:], fblob[r0:r0 + 128, :])
                    f32blk = sb.tile([128, C], F32, tag="f32blk")
                    nc.scalar.activation(f32blk[:], fblk[:], AF.Copy)
                    psum_t = sp.tile([C, 128], F32, tag="t")
                    nc.tensor.transpose(psum_t[:], f32blk[:], ident[:])
                    ftT = sb.tile([C, 128], F32, tag="ftT")
                    nc.scalar.activation(ftT[:], psum_t[:], AF.Copy)

                    psum_g = sp.tile([128, C], F32, tag="g")
                    nc.tensor.matmul(psum_g[:], ftT[:], wb[:], start=True, stop=False)
                    nc.tensor.matmul(psum_g[:], qTraw[:, r0:r0 + 128], wc[:],
                                     start=False, stop=False)
                    nc.tensor.matmul(psum_g[:], ones1[:], bsb[:], start=False, stop=True)
                    ghblk = sb.tile([128, C], F32, tag="ghblk")
                    nc.scalar.activation(ghblk[:], psum_g[:], AF.Copy)
                    nc.sync.dma_start(gh_loc[r0:r0 + 128, :], ghblk[:])

                    psum_a = sp.tile([128, C], F32, tag="a")
                    nc.tensor.matmul(psum_a[:], ftT[:], wd[:], start=True, stop=False)
                    nc.tensor.matmul(psum_a[:], qTraw[:, r0:r0 + 128], negwc[:],
                                     start=False, stop=True)
                    nc.scalar.activation(a_all[:, rb * C:(rb + 1) * C], psum_a[:],
                                         AF.Copy)

            # full 8192-row gather table
            nc.gpsimd.collective_compute(
                "AllGather", mybir.AluOpType.bypass, replica_groups=GROUPS4,
                ins=[gh_loc[:]], outs=[gh_d[:]])

            # --- main loop: per 128-query block ---
            with tc.tile_pool(name="mm_psum", bufs=6, space="PSUM") as mp, \
                 tc.tile_pool(name="srow", bufs=2) as spool, \
                 tc.tile_pool(name="small", bufs=4) as smp:
                for qb in range(n_qblk):
                    q0 = qb * 128
                    S = spool.tile([128, n_refs], F32, tag="S")
                    for ch in range(n_chunk):
                        c0 = ch * 512
                        psum_s = mp.tile([128, 512], F32, tag="s")
                        nc.tensor.matmul(psum_s[:], qT[:, q0:q0 + 128],
                                         refsT[:, c0:c0 + 512], start=True, stop=True)
                        nc.scalar.activation(S[:, c0:c0 + 512], psum_s[:], AF.Copy)

                    v = smp.tile([128, 16], F32, tag="v")
                    idx = smp.tile([128, 16], U32, tag="idx")
                    nc.vector.max(v[:, 0:8], S[:])
                    nc.vector.max_index(idx[:, 0:8], v[:, 0:8], S[:])
                    nc.vector.match_replace(S[:], v[:, 0:8], S[:], NEG)
                    nc.vector.max(v[:, 8:16], S[:])
                    nc.vector.max_index(idx[:, 8:16], v[:, 8:16], S[:])

                    nb = smp.tile([128, KNN * C], F32, tag="nb")
                    # HW indirect DMA consumes one offset per partition, so
                    # gather one 64-wide slab per neighbor k.
                    for k in range(KNN):
                        nc.gpsimd.indirect_dma_start(
                            out=nb[:, k * C:(k + 1) * C], out_offset=None,
                            in_=gh_d[:],
                            in_offset=IndirectOffsetOnAxis(ap=idx[:, k:k + 1], axis=0))

                    mx = smp.tile([128, C], F32, tag="mx")
                    nc.vector.tensor_reduce(
                        mx[:], nb[:].rearrange("p (k c) -> p c k", k=KNN),
                        axis=mybir.AxisListType.X, op=mybir.AluOpType.max)
                    nc.vector.tensor_add(mx[:], mx[:], a_all[:, qb * C:(qb + 1) * C])

                    # u8 quantize: rowmax -> scale 255/rm on the relu in one op
                    rm = smp.tile([128, 1], F32, tag="rm")
                    nc.vector.tensor_reduce(rm[:], mx[:], axis=mybir.AxisListType.X,
                                            op=mybir.AluOpType.max)
                    nc.vector.tensor_scalar_max(rm[:], rm[:], 1e-12)
                    inv = smp.tile([128, 1], F32, tag="inv")
                    nc.vector.reciprocal(inv[:], rm[:])
                    nc.vector.tensor_scalar_mul(inv[:], inv[:], 255.0)
                    srow = smp.tile([128, 1], F32, tag="srow")
                    nc.vector.tensor_scalar_mul(srow[:], rm[:], 1.0 / 255.0)
                    qb8 = smp.tile([128, C], U8, tag="qb8")
                    nc.scalar.activation(qb8[:], mx[:], AF.Relu, scale=inv[:])
                    nc.sync.dma_start(outq_loc[q0:q0 + 128, :], qb8[:])
                    nc.sync.dma_start(outs_loc[q0:q0 + 128, :], srow[:])

            # gather the batch output on every group member; the host reads
            # the group leader's shard
            nc.gpsimd.collective_compute(
                "AllGather", mybir.AluOpType.bypass, replica_groups=GROUPS4,
                ins=[outq_loc[:]], outs=[outq_bounce[:]])
            nc.gpsimd.collective_compute(
                "AllGather", mybir.AluOpType.bypass, replica_groups=GROUPS4,
                ins=[outs_loc[:]], outs=[outs_bounce[:]])
            nc.sync.dma_start(outq_d[:], outq_bounce[:])
            nc.sync.dma_start(outs_d[:], outs_bounce[:])

    return nc


_SCRATCH = {}
NHALF = NCORES // B            # cores per batch half (4)


def _pack_fblob_half(feat_b, s8, half):
    """f32 feat [N, C] (one batch) -> int8 codes (round to nearest), one
    buffer per half so a single device_put covers the half's 4 shards."""
    key = f"f{half}"
    bufs = _SCRATCH.get(key)
    if bufs is None:
        bufs = _SCRATCH[key] = (np.empty((N, C), np.float32),
                                np.empty((N, C), np.int8))
    tmp, codes = bufs
    np.multiply(feat_b, np.float32(1.0 / s8), out=tmp)
    np.rint(tmp, out=tmp)
    codes[:] = tmp                                  # f32 -> int8 cast copy
    return codes                                    # [8192, 64] int8


def _pack_cblob_all(coords_knn):
    """All cores' transposed coord blobs (both batches), core-major; uploaded
    with a single full-mesh put shared by both dispatches."""
    cb = _SCRATCH.get("c")
    if cb is None:
        cb = _SCRATCH["c"] = np.empty((NCORES * CT_ROWS, 64), np.float32)
    cbv = cb.reshape(NCORES, CT_ROWS, 64)
    for core in range(NCORES):
        h, q0 = divmod(core, NHALF)[0], (core % NHALF) * QPC
        cbv[core].reshape(3, QPC)[:] = coords_knn[h, q0:q0 + QPC].T
    return cb                                       # [8*96, 64] f32


def _pack_wblob(W, b, s8):
    """W (W1/W2 pre-scaled by the feat int8 scale) + b: core 0's shard only,
    shared by both batch dispatches."""
    wb_ = _SCRATCH.get("w")
    if wb_ is None:
        wb_ = _SCRATCH["w"] = np.empty((W_ROWS, 64), np.float32)
    wb_[:131] = W
    wb_[:2 * C] *= np.float32(s8)
    wb_[131] = b
    return wb_                                      # [132, 64] f32


_CACHE = {}


def _get_runner():
    if "runner" in _CACHE:
        return _CACHE["runner"]

    import jax
    from jax.sharding import Mesh, PartitionSpec
    from jax.experimental.shard_map import shard_map
    from concourse import bass2jax

    nc = build_kernel()
    nc.compile()
    bass2jax.install_neuronx_cc_hook()

    partition_name = nc.partition_id_tensor.name if nc.partition_id_tensor else None
    in_names, out_names, out_avals = [], [], []
    for alloc in nc.m.functions[0].allocations:
        if not isinstance(alloc, mybir.MemoryLocationSet):
            continue
        name = alloc.memorylocations[0].name
        if alloc.kind == "ExternalInput":
            if name != partition_name:
                in_names.append(name)
        elif alloc.kind == "ExternalOutput":
            out_names.append(name)
            out_avals.append(jax.core.ShapedArray(
                tuple(alloc.tensor_shape), mybir.dt.np(alloc.dtype)))
    assert in_names == ["fblob", "cblob", "wblob"] and \
        out_names == ["outq", "outs"], (in_names, out_names)
    n_params, n_outs = len(in_names), len(out_names)
    in_names_full = in_names + out_names + ([partition_name] if partition_name else [])
    donate = tuple(range(n_params, n_params + n_outs))

    def _body(*args):
        operands = list(args)
        if partition_name:
            operands.append(bass2jax.partition_id_tensor())
        return tuple(bass2jax._bass_exec_p.bind(
            *operands, out_avals=tuple(out_avals), in_names=tuple(in_names_full),
            out_names=tuple(out_names), lowering_input_output_aliases=(),
            sim_require_finite=True, sim_require_nnan=True, nc=nc))

    devices = jax.devices()[:NCORES]
    mesh = Mesh(np.asarray(devices), ("core",))
    sharded = jax.jit(
        shard_map(_body, mesh=mesh,
                  in_specs=(PartitionSpec("core"),) * (n_params + n_outs),
                  out_specs=(PartitionSpec("core"),) * n_outs, check_rep=False),
        donate_argnums=donate, keep_unused=True)
    shardspec = jax.sharding.NamedSharding(mesh, PartitionSpec("core"))
    # half-meshes used only to upload one batch's 4 real shards in one put
    half_specs = []
    for h in range(B):
        hmesh = Mesh(np.asarray(devices[h * NHALF:(h + 1) * NHALF]), ("c",))
        half_specs.append(jax.sharding.NamedSharding(hmesh, PartitionSpec("c")))
    # the idle half of each dispatch reads cached device-resident zeros
    # (never re-uploaded); all-zero inputs flow through the math safely
    zeros = {}
    for d in devices:
        zeros[d] = (
            jax.device_put(np.zeros((QPC, C), np.int8), d),
            jax.device_put(np.zeros((CT_ROWS, 64), np.float32), d),
            jax.device_put(np.zeros((W_ROWS, 64), np.float32), d),
        )
    _CACHE.update(runner=(sharded, out_avals, shardspec),
                  devices=devices, half_specs=half_specs, zeros=zeros)

    # Warm every dispatch path (incl. donated-Array args) so the caller's
    # first timed calls run the steady-state fast path.
    gw = _make_gw(np.zeros((W_ROWS, 64), np.float32))
    gc = jax.device_put(np.zeros((NCORES * CT_ROWS, 64), np.float32), shardspec)
    prev = []
    for h in range(B):
        gf = _assemble_fblob(np.zeros((N, C), np.float32), 1.0, h)
        ozs = tuple(np.zeros((NCORES * av.shape[0], av.shape[1]), av.dtype)
                    for av in out_avals)
        outs = sharded(gf, gc, gw, *ozs)
        np.asarray(min(outs[0].addressable_shards,
                       key=lambda s: s.index[0].start or 0).data)
        outs = sharded(gf, gc, gw, *outs)
        prev.append(outs)
    _CACHE["prev_out"] = prev
    return _CACHE["runner"]


def _make_gw(w_np):
    """Global wblob: real W on core 0, cached zero shards elsewhere."""
    import jax
    devices, zeros = _CACHE["devices"], _CACHE["zeros"]
    _, _, shardspec = _CACHE["runner"]
    dw = jax.device_put(w_np, devices[0])
    return jax.make_array_from_single_device_arrays(
        (NCORES * W_ROWS, 64), shardspec,
        [dw if d == devices[0] else zeros[d][2] for d in devices])


def _assemble_fblob(feat_b, s8, half):
    """Pack + assemble (single-threaded path, used by warmup)."""
    return _assemble_fblob_packed(_pack_fblob_half(feat_b, s8, half), half)


def _assemble_fblob_packed(codes, half):
    """Global fblob for one batch dispatch: the batch's 4 cores get real
    shards from one half-mesh put, the rest cached zeros."""
    import jax
    devices, zeros = _CACHE["devices"], _CACHE["zeros"]
    _, _, shardspec = _CACHE["runner"]
    df = jax.device_put(codes, _CACHE["half_specs"][half])
    fsh = {s.device: s.data for s in df.addressable_shards}
    return jax.make_array_from_single_device_arrays(
        (NCORES * QPC, C), shardspec,
        [fsh.get(d, zeros[d][0]) for d in devices])


def kernel(coords_knn, feat, W, b):
    import jax

    sharded, out_avals, shardspec = _get_runner()
    feat = np.ascontiguousarray(feat, np.float32)
    coords_knn = np.ascontiguousarray(coords_knn, np.float32)
    rows_per_core = out_avals[0].shape[0]           # 8192 (full batch rows)

    prev = _CACHE.pop("prev_out", None)
    if prev is None:
        prev = [tuple(np.zeros((NCORES * av.shape[0], av.shape[1]), av.dtype)
                      for av in out_avals) for _ in range(B)]

    # start the wire immediately with all coords (one full-mesh put shared
    # by both dispatches; tiny, lands before batch 0's feat finishes)
    gc = jax.device_put(_pack_cblob_all(coords_knn), shardspec)
    # worker thread runs the numpy-heavy scans/packs (GIL-releasing) ahead,
    # while the main thread issues puts/dispatches in order
    pool = _CACHE.get("pool")
    if pool is None:
        from concurrent.futures import ThreadPoolExecutor
        pool = _CACHE["pool"] = ThreadPoolExecutor(1)

    s8 = 1e-30
    for h in range(B):
        fh = feat[h]
        s8 = max(s8, float(max(fh.max(), -float(fh.min()))) / 127.0)
    # packs run ahead on the worker while the main thread issues gw/puts
    fut_packs = [pool.submit(_pack_fblob_half, feat[h], s8, h) for h in range(B)]
    gw = _make_gw(_pack_wblob(W, b, s8))            # shared by both dispatches

    # dispatch per batch: batch 0's download overlaps batch 1's upload
    all_outs, leaders = [], []
    for h in range(B):
        gf = _assemble_fblob_packed(fut_packs[h].result(), h)
        outs = sharded(gf, gc, gw, *prev[h])
        all_outs.append(outs)
        start = h * NHALF * rows_per_core           # leader core's shard
        shards = [next(s.data for s in o.addressable_shards
                       if (s.index[0].start or 0) == start) for o in outs]
        for sh in shards:
            try:
                sh.copy_to_host_async()  # queue the D2H behind the exec
            except Exception:
                pass
        leaders.append(shards)
    _CACHE["prev_out"] = all_outs  # donate into the next call

    out = np.empty((B, N, C), np.float32)
    for h in range(B):
        codes = np.asarray(leaders[h][0])           # [8192, 64] u8
        scales = np.asarray(leaders[h][1])          # [8192, 1] f32
        np.multiply(codes, scales, out=out[h], casting="unsafe")
    return out


# revision 64
# speedup vs baseline: 1.0358x; 1.0358x over previous
"""Trainium2 Bass kernel for nn_LocalAggBlock (KNN + gather + MLP + maxpool).

Math (exact refactoring of the reference):
  y[n,k] = relu(concat[f_n, f_nb-f_n, p_nb-p_n] @ W + b)
         = relu(a_n + gh[idx[n,k]])
  where a_n  = f_n @ (W1-W2) - p_n @ W3          (per query point)
        gh_m = f_m @ W2 + p_m @ W3 + b            (per reference point)
  out[n] = max_k y[n,k] = relu(a_n + max_k gh[idx[n,k]])   (relu/max commute,
           a_n constant over k)

  KNN ranking uses s'[n,m] = 2 p_n . p_m - ||p_m||^2 (larger = closer; the
  ||p_n||^2 term is constant per row and does not change the ranking).

Sharding: 8 cores = (batch b in 0..1) x (quarter of N).  Each core handles
2048 query points against all 8192 points of its batch.

Host<->device traffic over the axon tunnel is the bottleneck (~80ms RTT,
~95MB/s each way), so payloads are quantized and pipelined:
  - feat uploads as int8 (scale folded into W1/W2 host-side): 1MB up
  - W+b upload on core 0 only, once per call, shared by both batch
    dispatches; an on-device AllGather broadcasts them (other cores ship
    cached zero shards: no wire bytes)
  - output downloads as uint8 with a per-row f32 scale: ~1.06MB down
    (error budget: int8 feat + u8-row output simulates to 9.2e-3 rel err
    vs the 2e-2 gate)
  - one dispatch per batch, each touching all 8 cores but uploading real
    data only to its batch's 4 (cached zero shards elsewhere): batch 0's
    output download overlaps batch 1's input upload on the full-duplex
    tunnel, and host pack/unpack overlaps the wire as well
Full-batch tables are rebuilt on device with NeuronLink AllGathers
(coords, gh table, output).
"""

import numpy as np

import concourse.bacc as bacc
import concourse.bass as bass
import concourse.mybir as mybir
import concourse.tile as tile
from concourse.bass import IndirectOffsetOnAxis
from concourse.masks import make_identity

F32 = mybir.dt.float32
I8 = mybir.dt.int8
U8 = mybir.dt.uint8
U32 = mybir.dt.uint32
AF = mybir.ActivationFunctionType
NEG = -3.0e38

B, N, C = 2, 8192, 64
KNN = 16
NCORES = 8
QPC = B * N // NCORES          # queries per core (2048)
CT_ROWS = 96                   # coords^T (3, 2048) packed as (96, 64)
W_ROWS = 132                   # W (131, 64) + b (1, 64); core 0 only
GROUPS4 = [[0, 1, 2, 3], [4, 5, 6, 7]]   # per-batch replica groups
GROUPS8 = [list(range(NCORES))]


def build_kernel(n_refs=N, n_q=QPC):
    n_chunk = n_refs // 512    # ref chunks per query block
    n_qblk = n_q // 128        # query blocks
    n_grp = NCORES // B        # cores per batch

    nc = bacc.Bacc(None, target_bir_lowering=False)
    fblob = nc.dram_tensor("fblob", [n_q, C], I8, kind="ExternalInput")
    cblob = nc.dram_tensor("cblob", [CT_ROWS, 64], F32, kind="ExternalInput")
    wblob = nc.dram_tensor("wblob", [W_ROWS, 64], F32, kind="ExternalInput")
    outq_d = nc.dram_tensor("outq", [NCORES // B * n_q, C], U8,
                            kind="ExternalOutput")
    outs_d = nc.dram_tensor("outs", [NCORES // B * n_q, 1], F32,
                            kind="ExternalOutput")

    ct_loc = nc.dram_tensor("ct_loc", [CT_ROWS, 64], F32, kind="Internal")
    ct_all = nc.dram_tensor("ct_all", [n_grp * CT_ROWS, 64], F32, kind="Internal")
    w_loc = nc.dram_tensor("w_loc", [W_ROWS, 64], F32, kind="Internal")
    w_all = nc.dram_tensor("w_all", [NCORES * W_ROWS, 64], F32, kind="Internal")
    gh_loc = nc.dram_tensor("gh_loc", [n_q, C], F32, kind="Internal")
    gh_d = nc.dram_tensor("gh", [n_refs, C], F32, kind="Internal")
    outq_loc = nc.dram_tensor("outq_loc", [n_q, C], U8, kind="Internal")
    outs_loc = nc.dram_tensor("outs_loc", [n_q, 1], F32, kind="Internal")
    outq_bounce = nc.dram_tensor("outq_bounce", [n_grp * n_q, C], U8,
                                 kind="Internal")
    outs_bounce = nc.dram_tensor("outs_bounce", [n_grp * n_q, 1], F32,
                                 kind="Internal")

    with tile.TileContext(nc) as tc:
        with tc.tile_pool(name="persist", bufs=1) as pp:
            ident = pp.tile([128, 128], F32)
            make_identity(nc, ident[:])

            # --- coords AllGather (issued first; refsT consumes it later) ---
            nc.sync.dma_start(ct_loc[:], cblob[0:CT_ROWS, :])
            nc.gpsimd.collective_compute(
                "AllGather", mybir.AluOpType.bypass, replica_groups=GROUPS4,
                ins=[ct_loc[:]], outs=[ct_all[:]])
            # W+b broadcast: only core 0 uploads them, once per call (the
            # same wblob array feeds both batch dispatches; other cores ship
            # cached zero shards, no wire bytes); gather over all 8 cores
            # and read core 0's rows.
            nc.sync.dma_start(w_loc[:], wblob[:])
            nc.gpsimd.collective_compute(
                "AllGather", mybir.AluOpType.bypass, replica_groups=GROUPS8,
                ins=[w_loc[:]], outs=[w_all[:]])

            # --- weights (W1/W2 arrive pre-scaled by the feat int8 scale) ---
            wa = pp.tile([C, C], F32)
            wb = pp.tile([C, C], F32)
            wd = pp.tile([C, C], F32)     # W1 - W2
            wc = pp.tile([3, C], F32)
            negwc = pp.tile([3, C], F32)
            bsb = pp.tile([1, C], F32)
            ones1 = pp.tile([1, 128], F32)
            neg3 = pp.tile([3, 1], F32)
            nc.sync.dma_start(wa[:], w_all[0:64, :])
            nc.sync.dma_start(wb[:], w_all[64:128, :])
            nc.sync.dma_start(wc[:], w_all[128:131, :])
            nc.sync.dma_start(bsb[:], w_all[131:132, :])
            nc.vector.tensor_sub(wd[:], wa[:], wb[:])
            nc.vector.tensor_scalar_mul(negwc[:], wc[:], -1.0)
            nc.vector.memset(ones1[:], 1.0)
            nc.vector.memset(neg3[:], -1.0)

            # --- local query coords (transposed) ---
            qTraw = pp.tile([3, n_q], F32)      # raw query coords^T
            qT = pp.tile([4, n_q], F32)         # rows 0-2: 2*p_q^T, row 3: ones
            nc.sync.dma_start(
                qTraw[:], cblob[0:CT_ROWS, :].rearrange("(c r) f -> c (r f)", c=3))
            nc.vector.memset(qT[:], 1.0)  # row 3 stays 1.0
            nc.vector.tensor_scalar_mul(qT[0:3, :], qTraw[:], 2.0)

            # --- full-batch ref coords (transposed) from the AllGather ---
            refsT = pp.tile([4, n_refs], F32)   # rows 0-2: p^T, row 3: -||p||^2
            for g in range(n_grp):
                nc.sync.dma_start(
                    refsT[0:3, g * n_q:(g + 1) * n_q],
                    ct_all[g * CT_ROWS:(g + 1) * CT_ROWS, :].rearrange(
                        "(c r) f -> c (r f)", c=3))

            sq = pp.tile([3, n_refs], F32)
            nc.vector.tensor_mul(sq[:], refsT[0:3, :], refsT[0:3, :])

            a_all = pp.tile([128, n_qblk * C], F32)
            normrow = pp.tile([1, n_refs], F32)

            with tc.tile_pool(name="setup_psum", bufs=2, space="PSUM") as sp, \
                 tc.tile_pool(name="setup_sb", bufs=3) as sb:
                # row 3 of refsT: -(x^2+y^2+z^2) via PE partition-reduce
                for ch in range(n_chunk):
                    psum_n = sp.tile([1, 512], F32, tag="n")
                    nc.tensor.matmul(psum_n[:], neg3[:], sq[:, ch * 512:(ch + 1) * 512],
                                     start=True, stop=True)
                    nc.scalar.activation(normrow[0:1, ch * 512:(ch + 1) * 512],
                                         psum_n[:], AF.Copy)
                # compute engines can't start at partition 3; DMA can
                nc.sync.dma_start(refsT[3:4, :], normrow[:])

                # gh_loc[m] = f_m @ W2 + p_m @ W3 + b  and
                # a[n]      = f_n @ (W1-W2) - p_n @ W3   (same rows; share f^T)
                for rb in range(n_qblk):
                    r0 = rb * 128
                    fblk = sb.tile([128, C], I8, tag="fblk")
                    nc.sync.dma_start(fblk[:], fblob[r0:r0 + 128, :])
                    f32blk = sb.tile([128, C], F32, tag="f32blk")
                    nc.scalar.activation(f32blk[:], fblk[:], AF.Copy)
                    psum_t = sp.tile([C, 128], F32, tag="t")
                    nc.tensor.transpose(psum_t[:], f32blk[:], ident[:])
                    ftT = sb.tile([C, 128], F32, tag="ftT")
                    nc.scalar.activation(ftT[:], psum_t[:], AF.Copy)

                    psum_g = sp.tile([128, C], F32, tag="g")
                    nc.tensor.matmul(psum_g[:], ftT[:], wb[:], start=True, stop=False)
                    nc.tensor.matmul(psum_g[:], qTraw[:, r0:r0 + 128], wc[:],
                                     start=False, stop=False)
                    nc.tensor.matmul(psum_g[:], ones1[:], bsb[:], start=False, stop=True)
                    ghblk = sb.tile([128, C], F32, tag="ghblk")
                    nc.scalar.activation(ghblk[:], psum_g[:], AF.Copy)
                    nc.sync.dma_start(gh_loc[r0:r0 + 128, :], ghblk[:])

                    psum_a = sp.tile([128, C], F32, tag="a")
                    nc.tensor.matmul(psum_a[:], ftT[:], wd[:], start=True, stop=False)
                    nc.tensor.matmul(psum_a[:], qTraw[:, r0:r0 + 128], negwc[:],
                                     start=False, stop=True)
                    nc.scalar.activation(a_all[:, rb * C:(rb + 1) * C], psum_a[:],
                                         AF.Copy)

            # full 8192-row gather table
            nc.gpsimd.collective_compute(
                "AllGather", mybir.AluOpType.bypass, replica_groups=GROUPS4,
                ins=[gh_loc[:]], outs=[gh_d[:]])

            # --- main loop: per 128-query block ---
            with tc.tile_pool(name="mm_psum", bufs=6, space="PSUM") as mp, \
                 tc.tile_pool(name="srow", bufs=2) as spool, \
                 tc.tile_pool(name="small", bufs=4) as smp:
                for qb in range(n_qblk):
                    q0 = qb * 128
                    S = spool.tile([128, n_refs], F32, tag="S")
                    for ch in range(n_chunk):
                        c0 = ch * 512
                        psum_s = mp.tile([128, 512], F32, tag="s")
                        nc.tensor.matmul(psum_s[:], qT[:, q0:q0 + 128],
                                         refsT[:, c0:c0 + 512], start=True, stop=True)
                        nc.scalar.activation(S[:, c0:c0 + 512], psum_s[:], AF.Copy)

                    v = smp.tile([128, 16], F32, tag="v")
                    idx = smp.tile([128, 16], U32, tag="idx")
                    nc.vector.max(v[:, 0:8], S[:])
                    nc.vector.max_index(idx[:, 0:8], v[:, 0:8], S[:])
                    nc.vector.match_replace(S[:], v[:, 0:8], S[:], NEG)
                    nc.vector.max(v[:, 8:16], S[:])
                    nc.vector.max_index(idx[:, 8:16], v[:, 8:16], S[:])

                    nb = smp.tile([128, KNN * C], F32, tag="nb")
                    # HW indirect DMA consumes one offset per partition, so
                    # gather one 64-wide slab per neighbor k.
                    for k in range(KNN):
                        nc.gpsimd.indirect_dma_start(
                            out=nb[:, k * C:(k + 1) * C], out_offset=None,
                            in_=gh_d[:],
                            in_offset=IndirectOffsetOnAxis(ap=idx[:, k:k + 1], axis=0))

                    mx = smp.tile([128, C], F32, tag="mx")
                    nc.vector.tensor_reduce(
                        mx[:], nb[:].rearrange("p (k c) -> p c k", k=KNN),
                        axis=mybir.AxisListType.X, op=mybir.AluOpType.max)
                    nc.vector.tensor_add(mx[:], mx[:], a_all[:, qb * C:(qb + 1) * C])

                    # u8 quantize: rowmax -> scale 255/rm on the relu in one op
                    rm = smp.tile([128, 1], F32, tag="rm")
                    nc.vector.tensor_reduce(rm[:], mx[:], axis=mybir.AxisListType.X,
                                            op=mybir.AluOpType.max)
                    nc.vector.tensor_scalar_max(rm[:], rm[:], 1e-12)
                    inv = smp.tile([128, 1], F32, tag="inv")
                    nc.vector.reciprocal(inv[:], rm[:])
                    nc.vector.tensor_scalar_mul(inv[:], inv[:], 255.0)
                    srow = smp.tile([128, 1], F32, tag="srow")
                    nc.vector.tensor_scalar_mul(srow[:], rm[:], 1.0 / 255.0)
                    qb8 = smp.tile([128, C], U8, tag="qb8")
                    nc.scalar.activation(qb8[:], mx[:], AF.Relu, scale=inv[:])
                    nc.sync.dma_start(outq_loc[q0:q0 + 128, :], qb8[:])
                    nc.sync.dma_start(outs_loc[q0:q0 + 128, :], srow[:])

            # gather the batch output on every group member; the host reads
            # the group leader's shard
            nc.gpsimd.collective_compute(
                "AllGather", mybir.AluOpType.bypass, replica_groups=GROUPS4,
                ins=[outq_loc[:]], outs=[outq_bounce[:]])
            nc.gpsimd.collective_compute(
                "AllGather", mybir.AluOpType.bypass, replica_groups=GROUPS4,
                ins=[outs_loc[:]], outs=[outs_bounce[:]])
            nc.sync.dma_start(outq_d[:], outq_bounce[:])
            nc.sync.dma_start(outs_d[:], outs_bounce[:])

    return nc


_SCRATCH = {}
NHALF = NCORES // B            # cores per batch half (4)


def _pack_fblob_half(feat_b, s8, half):
    """f32 feat [N, C] (one batch) -> int8 codes (round to nearest), one
    buffer per half so a single device_put covers the half's 4 shards."""
    key = f"f{half}"
    bufs = _SCRATCH.get(key)
    if bufs is None:
        bufs = _SCRATCH[key] = (np.empty((N, C), np.float32),
                                np.empty((N, C), np.int8))
    tmp, codes = bufs
    np.multiply(feat_b, np.float32(1.0 / s8), out=tmp)
    np.rint(tmp, out=tmp)
    codes[:] = tmp                                  # f32 -> int8 cast copy
    return codes                                    # [8192, 64] int8


def _pack_cblob_all(coords_knn):
    """All cores' transposed coord blobs (both batches), core-major; uploaded
    with a single full-mesh put shared by both dispatches."""
    cb = _SCRATCH.get("c")
    if cb is None:
        cb = _SCRATCH["c"] = np.empty((NCORES * CT_ROWS, 64), np.float32)
    cbv = cb.reshape(NCORES, CT_ROWS, 64)
    for core in range(NCORES):
        h, q0 = divmod(core, NHALF)[0], (core % NHALF) * QPC
        cbv[core].reshape(3, QPC)[:] = coords_knn[h, q0:q0 + QPC].T
    return cb                                       # [8*96, 64] f32


def _pack_wblob(W, b, s8):
    """W (W1/W2 pre-scaled by the feat int8 scale) + b: core 0's shard only,
    shared by both batch dispatches."""
    wb_ = _SCRATCH.get("w")
    if wb_ is None:
        wb_ = _SCRATCH["w"] = np.empty((W_ROWS, 64), np.float32)
    wb_[:131] = W
    wb_[:2 * C] *= np.float32(s8)
    wb_[131] = b
    return wb_                                      # [132, 64] f32


_CACHE = {}


def _get_runner():
    if "runner" in _CACHE:
        return _CACHE["runner"]

    import jax
    from jax.sharding import Mesh, PartitionSpec
    from jax.experimental.shard_map import shard_map
    from concourse import bass2jax

    nc = build_kernel()
    nc.compile()
    bass2jax.install_neuronx_cc_hook()

    partition_name = nc.partition_id_tensor.name if nc.partition_id_tensor else None
    in_names, out_names, out_avals = [], [], []
    for alloc in nc.m.functions[0].allocations:
        if not isinstance(alloc, mybir.MemoryLocationSet):
            continue
        name = alloc.memorylocations[0].name
        if alloc.kind == "ExternalInput":
            if name != partition_name:
                in_names.append(name)
        elif alloc.kind == "ExternalOutput":
            out_names.append(name)
            out_avals.append(jax.core.ShapedArray(
                tuple(alloc.tensor_shape), mybir.dt.np(alloc.dtype)))
    assert in_names == ["fblob", "cblob", "wblob"] and \
        out_names == ["outq", "outs"], (in_names, out_names)
    n_params, n_outs = len(in_names), len(out_names)
    in_names_full = in_names + out_names + ([partition_name] if partition_name else [])
    donate = tuple(range(n_params, n_params + n_outs))

    def _body(*args):
        operands = list(args)
        if partition_name:
            operands.append(bass2jax.partition_id_tensor())
        return tuple(bass2jax._bass_exec_p.bind(
            *operands, out_avals=tuple(out_avals), in_names=tuple(in_names_full),
            out_names=tuple(out_names), lowering_input_output_aliases=(),
            sim_require_finite=True, sim_require_nnan=True, nc=nc))

    devices = jax.devices()[:NCORES]
    mesh = Mesh(np.asarray(devices), ("core",))
    sharded = jax.jit(
        shard_map(_body, mesh=mesh,
                  in_specs=(PartitionSpec("core"),) * (n_params + n_outs),
                  out_specs=(PartitionSpec("core"),) * n_outs, check_rep=False),
        donate_argnums=donate, keep_unused=True)
    shardspec = jax.sharding.NamedSharding(mesh, PartitionSpec("core"))
    # half-meshes used only to upload one batch's 4 real shards in one put
    half_specs = []
    for h in range(B):
        hmesh = Mesh(np.asarray(devices[h * NHALF:(h + 1) * NHALF]), ("c",))
        half_specs.append(jax.sharding.NamedSharding(hmesh, PartitionSpec("c")))
    # the idle half of each dispatch reads cached device-resident zeros
    # (never re-uploaded); all-zero inputs flow through the math safely
    zeros = {}
    for d in devices:
        zeros[d] = (
            jax.device_put(np.zeros((QPC, C), np.int8), d),
            jax.device_put(np.zeros((CT_ROWS, 64), np.float32), d),
            jax.device_put(np.zeros((W_ROWS, 64), np.float32), d),
        )
    _CACHE.update(runner=(sharded, out_avals, shardspec),
                  devices=devices, half_specs=half_specs, zeros=zeros)

    # Warm every dispatch path (incl. donated-Array args) so the caller's
    # first timed calls run the steady-state fast path.
    gw = _make_gw(np.zeros((W_ROWS, 64), np.float32))
    gc = jax.device_put(np.zeros((NCORES * CT_ROWS, 64), np.float32), shardspec)
    prev = []
    for h in range(B):
        gf = _assemble_fblob(np.zeros((N, C), np.float32), 1.0, h)
        ozs = tuple(np.zeros((NCORES * av.shape[0], av.shape[1]), av.dtype)
                    for av in out_avals)
        outs = sharded(gf, gc, gw, *ozs)
        np.asarray(min(outs[0].addressable_shards,
                       key=lambda s: s.index[0].start or 0).data)
        outs = sharded(gf, gc, gw, *outs)
        prev.append(outs)
    _CACHE["prev_out"] = prev
    return _CACHE["runner"]


def _make_gw(w_np):
    """Global wblob: real W on core 0, cached zero shards elsewhere."""
    import jax
    devices, zeros = _CACHE["devices"], _CACHE["zeros"]
    _, _, shardspec = _CACHE["runner"]
    dw = jax.device_put(w_np, devices[0])
    return jax.make_array_from_single_device_arrays(
        (NCORES * W_ROWS, 64), shardspec,
        [dw if d == devices[0] else zeros[d][2] for d in devices])


def _assemble_fblob(feat_b, s8, half):
    """Global fblob for one batch dispatch: the batch's 4 cores get real
    shards from one half-mesh put, the rest cached zeros."""
    import jax
    devices, zeros = _CACHE["devices"], _CACHE["zeros"]
    _, _, shardspec = _CACHE["runner"]
    df = jax.device_put(_pack_fblob_half(feat_b, s8, half),
                        _CACHE["half_specs"][half])
    fsh = {s.device: s.data for s in df.addressable_shards}
    return jax.make_array_from_single_device_arrays(
        (NCORES * QPC, C), shardspec,
        [fsh.get(d, zeros[d][0]) for d in devices])


def kernel(coords_knn, feat, W, b):
    import jax

    sharded, out_avals, shardspec = _get_runner()
    feat = np.ascontiguousarray(feat, np.float32)
    coords_knn = np.ascontiguousarray(coords_knn, np.float32)
    rows_per_core = out_avals[0].shape[0]           # 8192 (full batch rows)

    prev = _CACHE.pop("prev_out", None)
    if prev is None:
        prev = [tuple(np.zeros((NCORES * av.shape[0], av.shape[1]), av.dtype)
                      for av in out_avals) for _ in range(B)]

    # start the wire immediately with all coords (one full-mesh put shared
    # by both dispatches; tiny, lands before batch 0's feat finishes)
    gc = jax.device_put(_pack_cblob_all(coords_knn), shardspec)
    s8 = 1e-30
    for h in range(B):
        fh = feat[h]
        s8 = max(s8, float(max(fh.max(), -float(fh.min()))) / 127.0)
    gw = _make_gw(_pack_wblob(W, b, s8))            # shared by both dispatches

    # dispatch per batch: batch 0's download overlaps batch 1's upload
    all_outs, leaders = [], []
    for h in range(B):
        gf = _assemble_fblob(feat[h], s8, h)
        outs = sharded(gf, gc, gw, *prev[h])
        all_outs.append(outs)
        start = h * NHALF * rows_per_core           # leader core's shard
        shards = [next(s.data for s in o.addressable_shards
                       if (s.index[0].start or 0) == start) for o in outs]
        for sh in shards:
            try:
                sh.copy_to_host_async()  # queue the D2H behind the exec
            except Exception:
                pass
        leaders.append(shards)
    _CACHE["prev_out"] = all_outs  # donate into the next call

    out = np.empty((B, N, C), np.float32)
    for h in range(B):
        codes = np.asarray(leaders[h][0])           # [8192, 64] u8
        scales = np.asarray(leaders[h][1])          # [8192, 1] f32
        np.multiply(codes, scales, out=out[h], casting="unsafe")
    return out


# revision 65
# speedup vs baseline: 1.0598x; 1.0231x over previous
"""Trainium2 Bass kernel for nn_LocalAggBlock (KNN + gather + MLP + maxpool).

Math (exact refactoring of the reference):
  y[n,k] = relu(concat[f_n, f_nb-f_n, p_nb-p_n] @ W + b)
         = relu(a_n + gh[idx[n,k]])
  where a_n  = f_n @ (W1-W2) - p_n @ W3          (per query point)
        gh_m = f_m @ W2 + p_m @ W3 + b            (per reference point)
  out[n] = max_k y[n,k] = relu(a_n + max_k gh[idx[n,k]])   (relu/max commute,
           a_n constant over k)

  KNN ranking uses s'[n,m] = 2 p_n . p_m - ||p_m||^2 (larger = closer; the
  ||p_n||^2 term is constant per row and does not change the ranking).

Sharding: 8 cores = (batch b in 0..1) x (quarter of N).  Each core handles
2048 query points against all 8192 points of its batch.

Host<->device traffic over the axon tunnel is the bottleneck (~80ms RTT,
~95MB/s each way), so payloads are quantized and pipelined:
  - feat uploads as int8 (scale folded into W1/W2 host-side): 1MB up
  - W+b upload on core 0 only, once per call, shared by both batch
    dispatches; an on-device AllGather broadcasts them (other cores ship
    cached zero shards: no wire bytes)
  - output downloads as uint8 with a per-row f32 scale: ~1.06MB down
    (error budget: int8 feat + u8-row output simulates to 9.2e-3 rel err
    vs the 2e-2 gate)
  - one dispatch per batch, each touching all 8 cores but uploading real
    data only to its batch's 4 (cached zero shards elsewhere): batch 0's
    output download overlaps batch 1's input upload on the full-duplex
    tunnel, and host pack/unpack overlaps the wire as well
Full-batch tables are rebuilt on device with NeuronLink AllGathers
(coords, gh table, output).
"""

import numpy as np

import concourse.bacc as bacc
import concourse.bass as bass
import concourse.mybir as mybir
import concourse.tile as tile
from concourse.bass import IndirectOffsetOnAxis
from concourse.masks import make_identity

F32 = mybir.dt.float32
F16 = mybir.dt.float16
I8 = mybir.dt.int8
U8 = mybir.dt.uint8
U32 = mybir.dt.uint32
AF = mybir.ActivationFunctionType
NEG = -3.0e38

B, N, C = 2, 8192, 64
KNN = 16
NCORES = 8
QPC = B * N // NCORES          # queries per core (2048)
CT_ROWS = 96                   # coords^T (3, 2048) packed as (96, 64)
W_ROWS = 132                   # W (131, 64) + b (1, 64); core 0 only
GROUPS4 = [[0, 1, 2, 3], [4, 5, 6, 7]]   # per-batch replica groups
GROUPS8 = [list(range(NCORES))]


def build_kernel(n_refs=N, n_q=QPC):
    n_chunk = n_refs // 512    # ref chunks per query block
    n_qblk = n_q // 128        # query blocks
    n_grp = NCORES // B        # cores per batch

    nc = bacc.Bacc(None, target_bir_lowering=False)
    fblob = nc.dram_tensor("fblob", [n_q, C], I8, kind="ExternalInput")
    cblob = nc.dram_tensor("cblob", [CT_ROWS, 64], F32, kind="ExternalInput")
    wblob = nc.dram_tensor("wblob", [W_ROWS, 64], F32, kind="ExternalInput")
    outq_d = nc.dram_tensor("outq", [NCORES // B * n_q, C], U8,
                            kind="ExternalOutput")
    outs_d = nc.dram_tensor("outs", [NCORES // B * n_q, 1], F16,
                            kind="ExternalOutput")

    ct_loc = nc.dram_tensor("ct_loc", [CT_ROWS, 64], F32, kind="Internal")
    ct_all = nc.dram_tensor("ct_all", [n_grp * CT_ROWS, 64], F32, kind="Internal")
    w_loc = nc.dram_tensor("w_loc", [W_ROWS, 64], F32, kind="Internal")
    w_all = nc.dram_tensor("w_all", [NCORES * W_ROWS, 64], F32, kind="Internal")
    gh_loc = nc.dram_tensor("gh_loc", [n_q, C], F32, kind="Internal")
    gh_d = nc.dram_tensor("gh", [n_refs, C], F32, kind="Internal")
    outq_loc = nc.dram_tensor("outq_loc", [n_q, C], U8, kind="Internal")
    outs_loc = nc.dram_tensor("outs_loc", [n_q, 1], F16, kind="Internal")
    outq_bounce = nc.dram_tensor("outq_bounce", [n_grp * n_q, C], U8,
                                 kind="Internal")
    outs_bounce = nc.dram_tensor("outs_bounce", [n_grp * n_q, 1], F16,
                                 kind="Internal")

    with tile.TileContext(nc) as tc:
        with tc.tile_pool(name="persist", bufs=1) as pp:
            ident = pp.tile([128, 128], F32)
            make_identity(nc, ident[:])

            # --- coords AllGather (issued first; refsT consumes it later) ---
            nc.sync.dma_start(ct_loc[:], cblob[0:CT_ROWS, :])
            nc.gpsimd.collective_compute(
                "AllGather", mybir.AluOpType.bypass, replica_groups=GROUPS4,
                ins=[ct_loc[:]], outs=[ct_all[:]])
            # W+b broadcast: only core 0 uploads them, once per call (the
            # same wblob array feeds both batch dispatches; other cores ship
            # cached zero shards, no wire bytes); gather over all 8 cores
            # and read core 0's rows.
            nc.sync.dma_start(w_loc[:], wblob[:])
            nc.gpsimd.collective_compute(
                "AllGather", mybir.AluOpType.bypass, replica_groups=GROUPS8,
                ins=[w_loc[:]], outs=[w_all[:]])

            # --- weights (W1/W2 arrive pre-scaled by the feat int8 scale) ---
            wa = pp.tile([C, C], F32)
            wb = pp.tile([C, C], F32)
            wd = pp.tile([C, C], F32)     # W1 - W2
            wc = pp.tile([3, C], F32)
            negwc = pp.tile([3, C], F32)
            bsb = pp.tile([1, C], F32)
            ones1 = pp.tile([1, 128], F32)
            neg3 = pp.tile([3, 1], F32)
            nc.sync.dma_start(wa[:], w_all[0:64, :])
            nc.sync.dma_start(wb[:], w_all[64:128, :])
            nc.sync.dma_start(wc[:], w_all[128:131, :])
            nc.sync.dma_start(bsb[:], w_all[131:132, :])
            nc.vector.tensor_sub(wd[:], wa[:], wb[:])
            nc.vector.tensor_scalar_mul(negwc[:], wc[:], -1.0)
            nc.vector.memset(ones1[:], 1.0)
            nc.vector.memset(neg3[:], -1.0)

            # --- local query coords (transposed) ---
            qTraw = pp.tile([3, n_q], F32)      # raw query coords^T
            qT = pp.tile([4, n_q], F32)         # rows 0-2: 2*p_q^T, row 3: ones
            nc.sync.dma_start(
                qTraw[:], cblob[0:CT_ROWS, :].rearrange("(c r) f -> c (r f)", c=3))
            nc.vector.memset(qT[:], 1.0)  # row 3 stays 1.0
            nc.vector.tensor_scalar_mul(qT[0:3, :], qTraw[:], 2.0)

            # --- full-batch ref coords (transposed) from the AllGather ---
            refsT = pp.tile([4, n_refs], F32)   # rows 0-2: p^T, row 3: -||p||^2
            for g in range(n_grp):
                nc.sync.dma_start(
                    refsT[0:3, g * n_q:(g + 1) * n_q],
                    ct_all[g * CT_ROWS:(g + 1) * CT_ROWS, :].rearrange(
                        "(c r) f -> c (r f)", c=3))

            sq = pp.tile([3, n_refs], F32)
            nc.vector.tensor_mul(sq[:], refsT[0:3, :], refsT[0:3, :])

            a_all = pp.tile([128, n_qblk * C], F32)
            normrow = pp.tile([1, n_refs], F32)

            with tc.tile_pool(name="setup_psum", bufs=2, space="PSUM") as sp, \
                 tc.tile_pool(name="setup_sb", bufs=3) as sb:
                # row 3 of refsT: -(x^2+y^2+z^2) via PE partition-reduce
                for ch in range(n_chunk):
                    psum_n = sp.tile([1, 512], F32, tag="n")
                    nc.tensor.matmul(psum_n[:], neg3[:], sq[:, ch * 512:(ch + 1) * 512],
                                     start=True, stop=True)
                    nc.scalar.activation(normrow[0:1, ch * 512:(ch + 1) * 512],
                                         psum_n[:], AF.Copy)
                # compute engines can't start at partition 3; DMA can
                nc.sync.dma_start(refsT[3:4, :], normrow[:])

                # gh_loc[m] = f_m @ W2 + p_m @ W3 + b  and
                # a[n]      = f_n @ (W1-W2) - p_n @ W3   (same rows; share f^T)
                for rb in range(n_qblk):
                    r0 = rb * 128
                    fblk = sb.tile([128, C], I8, tag="fblk")
                    nc.sync.dma_start(fblk[:], fblob[r0:r0 + 128, :])
                    f32blk = sb.tile([128, C], F32, tag="f32blk")
                    nc.scalar.activation(f32blk[:], fblk[:], AF.Copy)
                    psum_t = sp.tile([C, 128], F32, tag="t")
                    nc.tensor.transpose(psum_t[:], f32blk[:], ident[:])
                    ftT = sb.tile([C, 128], F32, tag="ftT")
                    nc.scalar.activation(ftT[:], psum_t[:], AF.Copy)

                    psum_g = sp.tile([128, C], F32, tag="g")
                    nc.tensor.matmul(psum_g[:], ftT[:], wb[:], start=True, stop=False)
                    nc.tensor.matmul(psum_g[:], qTraw[:, r0:r0 + 128], wc[:],
                                     start=False, stop=False)
                    nc.tensor.matmul(psum_g[:], ones1[:], bsb[:], start=False, stop=True)
                    ghblk = sb.tile([128, C], F32, tag="ghblk")
                    nc.scalar.activation(ghblk[:], psum_g[:], AF.Copy)
                    nc.sync.dma_start(gh_loc[r0:r0 + 128, :], ghblk[:])

                    psum_a = sp.tile([128, C], F32, tag="a")
                    nc.tensor.matmul(psum_a[:], ftT[:], wd[:], start=True, stop=False)
                    nc.tensor.matmul(psum_a[:], qTraw[:, r0:r0 + 128], negwc[:],
                                     start=False, stop=True)
                    nc.scalar.activation(a_all[:, rb * C:(rb + 1) * C], psum_a[:],
                                         AF.Copy)

            # full 8192-row gather table
            nc.gpsimd.collective_compute(
                "AllGather", mybir.AluOpType.bypass, replica_groups=GROUPS4,
                ins=[gh_loc[:]], outs=[gh_d[:]])

            # --- main loop: per 128-query block ---
            with tc.tile_pool(name="mm_psum", bufs=6, space="PSUM") as mp, \
                 tc.tile_pool(name="srow", bufs=2) as spool, \
                 tc.tile_pool(name="small", bufs=4) as smp:
                for qb in range(n_qblk):
                    q0 = qb * 128
                    S = spool.tile([128, n_refs], F32, tag="S")
                    for ch in range(n_chunk):
                        c0 = ch * 512
                        psum_s = mp.tile([128, 512], F32, tag="s")
                        nc.tensor.matmul(psum_s[:], qT[:, q0:q0 + 128],
                                         refsT[:, c0:c0 + 512], start=True, stop=True)
                        nc.scalar.activation(S[:, c0:c0 + 512], psum_s[:], AF.Copy)

                    v = smp.tile([128, 16], F32, tag="v")
                    idx = smp.tile([128, 16], U32, tag="idx")
                    nc.vector.max(v[:, 0:8], S[:])
                    nc.vector.max_index(idx[:, 0:8], v[:, 0:8], S[:])
                    nc.vector.match_replace(S[:], v[:, 0:8], S[:], NEG)
                    nc.vector.max(v[:, 8:16], S[:])
                    nc.vector.max_index(idx[:, 8:16], v[:, 8:16], S[:])

                    nb = smp.tile([128, KNN * C], F32, tag="nb")
                    # HW indirect DMA consumes one offset per partition, so
                    # gather one 64-wide slab per neighbor k.
                    for k in range(KNN):
                        nc.gpsimd.indirect_dma_start(
                            out=nb[:, k * C:(k + 1) * C], out_offset=None,
                            in_=gh_d[:],
                            in_offset=IndirectOffsetOnAxis(ap=idx[:, k:k + 1], axis=0))

                    mx = smp.tile([128, C], F32, tag="mx")
                    nc.vector.tensor_reduce(
                        mx[:], nb[:].rearrange("p (k c) -> p c k", k=KNN),
                        axis=mybir.AxisListType.X, op=mybir.AluOpType.max)
                    nc.vector.tensor_add(mx[:], mx[:], a_all[:, qb * C:(qb + 1) * C])

                    # u8 quantize: rowmax -> scale 255/rm on the relu in one op
                    rm = smp.tile([128, 1], F32, tag="rm")
                    nc.vector.tensor_reduce(rm[:], mx[:], axis=mybir.AxisListType.X,
                                            op=mybir.AluOpType.max)
                    nc.vector.tensor_scalar_max(rm[:], rm[:], 1e-12)
                    inv = smp.tile([128, 1], F32, tag="inv")
                    nc.vector.reciprocal(inv[:], rm[:])
                    nc.vector.tensor_scalar_mul(inv[:], inv[:], 255.0)
                    srow = smp.tile([128, 1], F16, tag="srow")
                    nc.vector.tensor_scalar_mul(srow[:], rm[:], 1.0 / 255.0)
                    qb8 = smp.tile([128, C], U8, tag="qb8")
                    nc.scalar.activation(qb8[:], mx[:], AF.Relu, scale=inv[:])
                    nc.sync.dma_start(outq_loc[q0:q0 + 128, :], qb8[:])
                    nc.sync.dma_start(outs_loc[q0:q0 + 128, :], srow[:])

            # gather the batch output on every group member; the host reads
            # the group leader's shard
            nc.gpsimd.collective_compute(
                "AllGather", mybir.AluOpType.bypass, replica_groups=GROUPS4,
                ins=[outq_loc[:]], outs=[outq_bounce[:]])
            nc.gpsimd.collective_compute(
                "AllGather", mybir.AluOpType.bypass, replica_groups=GROUPS4,
                ins=[outs_loc[:]], outs=[outs_bounce[:]])
            nc.sync.dma_start(outq_d[:], outq_bounce[:])
            nc.sync.dma_start(outs_d[:], outs_bounce[:])

    return nc


_SCRATCH = {}
NHALF = NCORES // B            # cores per batch half (4)


def _pack_fblob_half(feat_b, s8, half):
    """f32 feat [N, C] (one batch) -> int8 codes (round to nearest), one
    buffer per half so a single device_put covers the half's 4 shards."""
    key = f"f{half}"
    bufs = _SCRATCH.get(key)
    if bufs is None:
        bufs = _SCRATCH[key] = (np.empty((N, C), np.float32),
                                np.empty((N, C), np.int8))
    tmp, codes = bufs
    np.multiply(feat_b, np.float32(1.0 / s8), out=tmp)
    np.rint(tmp, out=tmp)
    codes[:] = tmp                                  # f32 -> int8 cast copy
    return codes                                    # [8192, 64] int8


def _pack_cblob_all(coords_knn):
    """All cores' transposed coord blobs (both batches), core-major; uploaded
    with a single full-mesh put shared by both dispatches."""
    cb = _SCRATCH.get("c")
    if cb is None:
        cb = _SCRATCH["c"] = np.empty((NCORES * CT_ROWS, 64), np.float32)
    cbv = cb.reshape(NCORES, CT_ROWS, 64)
    for core in range(NCORES):
        h, q0 = divmod(core, NHALF)[0], (core % NHALF) * QPC
        cbv[core].reshape(3, QPC)[:] = coords_knn[h, q0:q0 + QPC].T
    return cb                                       # [8*96, 64] f32


def _pack_wblob(W, b, s8):
    """W (W1/W2 pre-scaled by the feat int8 scale) + b: core 0's shard only,
    shared by both batch dispatches."""
    wb_ = _SCRATCH.get("w")
    if wb_ is None:
        wb_ = _SCRATCH["w"] = np.empty((W_ROWS, 64), np.float32)
    wb_[:131] = W
    wb_[:2 * C] *= np.float32(s8)
    wb_[131] = b
    return wb_                                      # [132, 64] f32


_CACHE = {}


def _get_runner():
    if "runner" in _CACHE:
        return _CACHE["runner"]

    import jax
    from jax.sharding import Mesh, PartitionSpec
    from jax.experimental.shard_map import shard_map
    from concourse import bass2jax

    nc = build_kernel()
    nc.compile()
    bass2jax.install_neuronx_cc_hook()

    partition_name = nc.partition_id_tensor.name if nc.partition_id_tensor else None
    in_names, out_names, out_avals = [], [], []
    for alloc in nc.m.functions[0].allocations:
        if not isinstance(alloc, mybir.MemoryLocationSet):
            continue
        name = alloc.memorylocations[0].name
        if alloc.kind == "ExternalInput":
            if name != partition_name:
                in_names.append(name)
        elif alloc.kind == "ExternalOutput":
            out_names.append(name)
            out_avals.append(jax.core.ShapedArray(
                tuple(alloc.tensor_shape), mybir.dt.np(alloc.dtype)))
    assert in_names == ["fblob", "cblob", "wblob"] and \
        out_names == ["outq", "outs"], (in_names, out_names)
    n_params, n_outs = len(in_names), len(out_names)
    in_names_full = in_names + out_names + ([partition_name] if partition_name else [])
    donate = tuple(range(n_params, n_params + n_outs))

    def _body(*args):
        operands = list(args)
        if partition_name:
            operands.append(bass2jax.partition_id_tensor())
        return tuple(bass2jax._bass_exec_p.bind(
            *operands, out_avals=tuple(out_avals), in_names=tuple(in_names_full),
            out_names=tuple(out_names), lowering_input_output_aliases=(),
            sim_require_finite=True, sim_require_nnan=True, nc=nc))

    devices = jax.devices()[:NCORES]
    mesh = Mesh(np.asarray(devices), ("core",))
    sharded = jax.jit(
        shard_map(_body, mesh=mesh,
                  in_specs=(PartitionSpec("core"),) * (n_params + n_outs),
                  out_specs=(PartitionSpec("core"),) * n_outs, check_rep=False),
        donate_argnums=donate, keep_unused=True)
    shardspec = jax.sharding.NamedSharding(mesh, PartitionSpec("core"))
    # half-meshes used only to upload one batch's 4 real shards in one put
    half_specs = []
    for h in range(B):
        hmesh = Mesh(np.asarray(devices[h * NHALF:(h + 1) * NHALF]), ("c",))
        half_specs.append(jax.sharding.NamedSharding(hmesh, PartitionSpec("c")))
    # the idle half of each dispatch reads cached device-resident zeros
    # (never re-uploaded); all-zero inputs flow through the math safely
    zeros = {}
    for d in devices:
        zeros[d] = (
            jax.device_put(np.zeros((QPC, C), np.int8), d),
            jax.device_put(np.zeros((CT_ROWS, 64), np.float32), d),
            jax.device_put(np.zeros((W_ROWS, 64), np.float32), d),
        )
    _CACHE.update(runner=(sharded, out_avals, shardspec),
                  devices=devices, half_specs=half_specs, zeros=zeros)

    # Warm every dispatch path (incl. donated-Array args) so the caller's
    # first timed calls run the steady-state fast path.
    gw = _make_gw(np.zeros((W_ROWS, 64), np.float32))
    gc = jax.device_put(np.zeros((NCORES * CT_ROWS, 64), np.float32), shardspec)
    prev = []
    for h in range(B):
        gf = _assemble_fblob(np.zeros((N, C), np.float32), 1.0, h)
        ozs = tuple(np.zeros((NCORES * av.shape[0], av.shape[1]), av.dtype)
                    for av in out_avals)
        outs = sharded(gf, gc, gw, *ozs)
        np.asarray(min(outs[0].addressable_shards,
                       key=lambda s: s.index[0].start or 0).data)
        outs = sharded(gf, gc, gw, *outs)
        prev.append(outs)
    _CACHE["prev_out"] = prev
    return _CACHE["runner"]


def _make_gw(w_np):
    """Global wblob: real W on core 0, cached zero shards elsewhere."""
    import jax
    devices, zeros = _CACHE["devices"], _CACHE["zeros"]
    _, _, shardspec = _CACHE["runner"]
    dw = jax.device_put(w_np, devices[0])
    return jax.make_array_from_single_device_arrays(
        (NCORES * W_ROWS, 64), shardspec,
        [dw if d == devices[0] else zeros[d][2] for d in devices])


def _assemble_fblob(feat_b, s8, half):
    """Global fblob for one batch dispatch: the batch's 4 cores get real
    shards from one half-mesh put, the rest cached zeros."""
    import jax
    devices, zeros = _CACHE["devices"], _CACHE["zeros"]
    _, _, shardspec = _CACHE["runner"]
    df = jax.device_put(_pack_fblob_half(feat_b, s8, half),
                        _CACHE["half_specs"][half])
    fsh = {s.device: s.data for s in df.addressable_shards}
    return jax.make_array_from_single_device_arrays(
        (NCORES * QPC, C), shardspec,
        [fsh.get(d, zeros[d][0]) for d in devices])


def kernel(coords_knn, feat, W, b):
    import jax

    sharded, out_avals, shardspec = _get_runner()
    feat = np.ascontiguousarray(feat, np.float32)
    coords_knn = np.ascontiguousarray(coords_knn, np.float32)
    rows_per_core = out_avals[0].shape[0]           # 8192 (full batch rows)

    prev = _CACHE.pop("prev_out", None)
    if prev is None:
        prev = [tuple(np.zeros((NCORES * av.shape[0], av.shape[1]), av.dtype)
                      for av in out_avals) for _ in range(B)]

    # start the wire immediately with all coords (one full-mesh put shared
    # by both dispatches; tiny, lands before batch 0's feat finishes)
    gc = jax.device_put(_pack_cblob_all(coords_knn), shardspec)
    s8 = 1e-30
    for h in range(B):
        fh = feat[h]
        s8 = max(s8, float(max(fh.max(), -float(fh.min()))) / 127.0)
    gw = _make_gw(_pack_wblob(W, b, s8))            # shared by both dispatches

    # dispatch per batch: batch 0's download overlaps batch 1's upload
    all_outs, leaders = [], []
    for h in range(B):
        gf = _assemble_fblob(feat[h], s8, h)
        outs = sharded(gf, gc, gw, *prev[h])
        all_outs.append(outs)
        start = h * NHALF * rows_per_core           # leader core's shard
        shards = [next(s.data for s in o.addressable_shards
                       if (s.index[0].start or 0) == start) for o in outs]
        for sh in shards:
            try:
                sh.copy_to_host_async()  # queue the D2H behind the exec
            except Exception:
                pass
        leaders.append(shards)
    _CACHE["prev_out"] = all_outs  # donate into the next call

    out = np.empty((B, N, C), np.float32)
    for h in range(B):
        codes = np.asarray(leaders[h][0])           # [8192, 64] u8
        scales = np.asarray(leaders[h][1]).astype(np.float32)  # [8192, 1]
        np.multiply(codes, scales, out=out[h], casting="unsafe")
    return out
